# revision 1
# baseline (speedup 1.0000x reference)
"""GNN message passing (GCNConv -> global mean pool -> dense softmax) on 8 TRN2 cores.

Sharding: graphs are partitioned by seg_ids so each core owns 8 whole graphs
(a contiguous node range).  Edges are routed to the core that owns their
destination node; each core gathers source-node features (raw x rows, 512B)
straight from HBM with dma_gather, aggregates messages into per-window PSUM
tiles via one-hot matmuls on the TensorEngine, then runs the node GEMM,
segment-mean pooling and the dense softmax head locally.  No collectives.

All data-dependent structure (gather indices, one-hot selectors, pooling
matrix) is carried in per-core input arrays so one SPMD program serves all
8 cores.
"""

import sys

sys.path.insert(0, "/opt/trn_rl_repo")

import numpy as np

import concourse.bass as bass
import concourse.bacc as bacc
import concourse.mybir as mybir
import concourse.tile as tile
from concourse.bass_utils import run_bass_kernel_spmd

N_CORES = 8
N_GRAPHS = 64
G_PER_CORE = N_GRAPHS // N_CORES
P = 128          # lanes per chunk / dst slots per window
CPC = int(__import__("os").environ.get("KCPC", "48"))  # chunks per gather call
N_QUEUES = int(__import__("os").environ.get("KQ", "4"))
KOH = __import__("os").environ.get("KOH", "dve")      # dve | ship
KOHDT = __import__("os").environ.get("KOHDT", "bf16")  # bf16 | fp32
MAX_I16 = 32768


def _balance_windows(deg4, W, slack_slots):
    """Assign each dst (rows of deg4 [n,4]) to one of W windows, <=128 dsts per
    window, minimizing the max per-group edge load.  Returns window_of_dst."""
    n = deg4.shape[0]
    order = np.argsort(-deg4.sum(1), kind="stable")
    load = np.zeros((W, 4), np.int64)
    slots = np.zeros(W, np.int64)
    win = np.zeros(n, np.int64)
    for d in order:
        free = slots < P
        # score: resulting max group load of each candidate window
        cand = (load[free] + deg4[d]).max(1)
        wsel = np.flatnonzero(free)[np.argmin(cand)]
        win[d] = wsel
        load[wsel] += deg4[d]
        slots[wsel] += 1
    return win, load


def _prep_core(es, dl, ew, n_loc, W, gbase, n_groups):
    """Per-core schedule: window/slot assignment and per-(group,window) edge
    blocks.  Returns dict with row permutation + per-edge schedule keys."""
    g = (es // gbase).astype(np.int64)
    deg4 = np.zeros((max(n_loc, 1), 4), np.int64)
    np.add.at(deg4, (dl, g), 1)
    win, load = _balance_windows(deg4, W, W * P - n_loc)
    # slot within window, in assignment-independent (stable by dst id) order
    order = np.argsort(win * (10 ** 9) + np.arange(max(n_loc, 1)), kind="stable")
    slot_of = np.zeros(max(n_loc, 1), np.int64)
    for w in range(W):
        members = np.flatnonzero(win == w)
        slot_of[members] = np.arange(len(members))
    row_of = win * P + slot_of
    q_req = int(np.ceil(load.max() / P)) if load.size else 1
    return {
        "g": g, "win": win[dl], "slot": slot_of[dl], "row_of": row_of,
        "q_req": max(q_req, 1),
    }


def _build_schedule(core, q, W, n_loc):
    """Fill offs16 / dsub / wv [P, C] arrays for one core given global q."""
    g, win, slot = core["g"], core["win"], core["slot"]
    C = 4 * W * q
    offs = np.zeros((P, C), np.int16)
    dsub = np.full((P, C), -1.0, np.float32)
    wv = np.zeros((P, C), np.float32)
    # order edges by (g, w, slot)
    key = (g * W + win) * P + slot
    order = np.argsort(key, kind="stable")
    gs, ws = g[order], win[order]
    # position within each (g,w) block
    blk = gs * W + ws
    changes = np.r_[True, blk[1:] != blk[:-1]]
    block_start = np.maximum.accumulate(np.where(changes, np.arange(len(blk)), 0))
    pos = np.arange(len(blk)) - block_start
    k = (gs * W + ws) * q + pos // P
    p = pos % P
    if np.any(pos // P >= q):
        raise RuntimeError("window overflow: q too small")
    offs[p, k] = (core["es_sorted"][order] - gs * core["gbase"]).astype(np.int16)
    dsub[p, k] = slot[order].astype(np.float32)
    wv[p, k] = core["ew_sorted"][order]
    ohw = None
    if KOH == "ship":
        import ml_dtypes
        odt = ml_dtypes.bfloat16 if KOHDT == "bf16" else np.float32
        ohw = np.zeros((P, C, P), odt)
        ohw[p, k, slot[order]] = core["ew_sorted"][order].astype(odt)
    return offs, dsub, wv, ohw


def _wrap_idx(offs, W, q):
    """Pack per-call int16 index lists in the HW wrap-16 layout, 8x replicated.
    Calls are group-pure spans of up to CPC chunks.  Returns [128, total/16]."""
    per_g = W * q
    cols = []
    for g in range(4):
        m = 0
        while m < per_g:
            nch = min(CPC, per_g - m)
            k0 = g * per_g + m
            ii = offs[:, k0:k0 + nch].T.reshape(-1)  # i = chunk*128 + p
            cols.append(ii.reshape(-1, 16).T)        # [16, n/16]
            m += nch
    arr = np.concatenate(cols, axis=1)
    return np.tile(arr, (8, 1)).astype(np.int16)


def _prepare(x, edge_src, edge_dst, edge_weight, seg_ids, W1, b1, W2, b2):
    N = x.shape[0]
    n_groups = 4
    gbase = int(np.ceil(N / n_groups))
    assert gbase <= MAX_I16
    bounds = np.searchsorted(seg_ids, np.arange(0, N_GRAPHS + 1, G_PER_CORE))
    n_locs = np.diff(bounds)
    W = int(np.ceil(n_locs.max() / P))
    core_of_edge = np.searchsorted(bounds, edge_dst, side="right") - 1

    cores = []
    for c in range(N_CORES):
        m = core_of_edge == c
        es, ed, ew = edge_src[m], edge_dst[m] - bounds[c], edge_weight[m]
        info = _prep_core(es, ed, ew, int(n_locs[c]), W, gbase, n_groups)
        info.update(es_sorted=es, ew_sorted=ew, gbase=gbase)
        cores.append(info)
    q = max(ci["q_req"] for ci in cores)
    C = 4 * W * q

    iota = np.tile(np.arange(P, dtype=np.float32), (P, 1))
    ident = np.eye(P, dtype=np.float32)
    b1b = np.tile(b1[None, :], (P, 1)).astype(np.float32)
    b2b = np.tile(b2[None, :], (G_PER_CORE, 1)).astype(np.float32)

    in_maps = []
    for c in range(N_CORES):
        ci = cores[c]
        offs, dsub, wv, ohw = _build_schedule(ci, q, W, int(n_locs[c]))
        idx16 = _wrap_idx(offs, W, q)
        # pooling matrix in SBUF layout [128, W*G_PER_CORE]:
        # row r = w*128+s of the padded node space -> local graph one-hot
        pool = np.zeros((P, W, G_PER_CORE), np.float32)
        segs_loc = seg_ids[bounds[c]:bounds[c + 1]] - c * G_PER_CORE
        rows = ci["row_of"][:n_locs[c]]
        pool[rows % P, rows // P, segs_loc] = 1.0
        cnts = np.bincount(segs_loc, minlength=G_PER_CORE).astype(np.float32)
        invc = np.tile((1.0 / np.maximum(cnts, 1.0))[None, :], (64, 1)).astype(np.float32)
        if KOH == "ship" and KOHDT == "bf16":
            import ml_dtypes
            x_ship = np.ascontiguousarray(x.astype(ml_dtypes.bfloat16))
        else:
            x_ship = np.ascontiguousarray(x, np.float32)
        in_maps.append({
            "x": x_ship,
            "idx16": idx16,
            "dsub": dsub, "wv": wv,
            "pool": pool.reshape(P, W * G_PER_CORE),
            "invc": invc,
            "W1": np.ascontiguousarray(W1, np.float32),
            "b1b": b1b,
            "W2": np.ascontiguousarray(W2, np.float32),
            "b2b": b2b,
            "iota": iota, "ident": ident,
            **({"ohw": ohw.reshape(P, -1)} if ohw is not None else {}),
        })
    meta = {"N": N, "W": W, "q": q, "C": C, "gbase": gbase,
            "idx_cols": in_maps[0]["idx16"].shape[1]}
    return in_maps, meta


def _build_program(meta):
    N, W, q, gbase = meta["N"], meta["W"], meta["q"], meta["gbase"]
    C = meta["C"]
    f32 = mybir.dt.float32
    nc = bacc.Bacc("TRN2", target_bir_lowering=False, debug=False,
                   num_devices=N_CORES, num_swdge_queues=N_QUEUES)
    gdt = (mybir.dt.bfloat16 if (KOH == "ship" and KOHDT == "bf16") else f32)
    x = nc.declare_dram_parameter("x", [N, 128], gdt, isOutput=False)
    idx16 = nc.declare_dram_parameter("idx16", [128, meta["idx_cols"]], mybir.dt.int16, isOutput=False)
    dsub = nc.declare_dram_parameter("dsub", [128, C], f32, isOutput=False)
    wv = nc.declare_dram_parameter("wv", [128, C], f32, isOutput=False)
    pool_p = nc.declare_dram_parameter("pool", [128, W * G_PER_CORE], f32, isOutput=False)
    invc_p = nc.declare_dram_parameter("invc", [64, G_PER_CORE], f32, isOutput=False)
    W1_p = nc.declare_dram_parameter("W1", [128, 64], f32, isOutput=False)
    b1b_p = nc.declare_dram_parameter("b1b", [128, 64], f32, isOutput=False)
    W2_p = nc.declare_dram_parameter("W2", [64, 4], f32, isOutput=False)
    b2b_p = nc.declare_dram_parameter("b2b", [G_PER_CORE, 4], f32, isOutput=False)
    iota_p = nc.declare_dram_parameter("iota", [128, 128], f32, isOutput=False)
    odt = mybir.dt.bfloat16 if KOHDT == "bf16" else f32
    ohw_p = (nc.declare_dram_parameter("ohw", [128, C * 128], odt, isOutput=False)
             if KOH == "ship" else None)
    ident_p = nc.declare_dram_parameter("ident", [128, 128], f32, isOutput=False)
    probs = nc.declare_dram_parameter("probs", [G_PER_CORE, 4], f32, isOutput=True)

    per_g = W * q
    with tile.TileContext(nc) as tc:
        with tc.tile_pool(name="const", bufs=1) as cp, \
             tc.tile_pool(name="agg", bufs=1) as aggp, \
             tc.tile_pool(name="stream", bufs=1) as sp:
            iota_t = cp.tile([128, 128], f32)
            nc.sync.dma_start(iota_t[:], iota_p[:])
            ident_t = cp.tile([128, 128], f32)
            nc.sync.dma_start(ident_t[:], ident_p[:])
            w1_t = cp.tile([128, 64], f32)
            nc.sync.dma_start(w1_t[:], W1_p[:])
            b1b_t = cp.tile([128, 64], f32)
            nc.sync.dma_start(b1b_t[:], b1b_p[:])
            w2_t = cp.tile([64, 4], f32)
            nc.sync.dma_start(w2_t[:], W2_p[:])
            b2b_t = cp.tile([G_PER_CORE, 4], f32)
            nc.sync.dma_start(b2b_t[:], b2b_p[:])
            invc_t = cp.tile([64, G_PER_CORE], f32)
            nc.sync.dma_start(invc_t[:], invc_p[:])
            pool_t = cp.tile([128, W, G_PER_CORE], f32)
            nc.sync.dma_start(pool_t[:], pool_p[:].rearrange("p (w f) -> p w f", f=G_PER_CORE))
            dsub_t = sp.tile([128, C], f32)
            nc.sync.dma_start(dsub_t[:], dsub[:])
            wv_t = sp.tile([128, C], f32)
            nc.sync.dma_start(wv_t[:], wv[:])
            agg_t = aggp.tile([128, W, 128], f32)

            # ---- main loop: gather + one-hot matmuls into window PSUMs ----
            import os as _os
            _os2 = _os
            KREP = int(_os.environ.get("KREP", "1"))
            KABL = _os.environ.get("KABL", "full")  # full|gather|nodve|nope|dve
            with tc.tile_pool(name="gbuf", bufs=int(_os2.environ.get("KGB", "4"))) as gp, \
                 tc.tile_pool(name="idxs", bufs=4) as ixp, \
                 tc.tile_pool(name="oh", bufs=(8 if KOH == "dve" else 1)) as ohp, \
                 tc.tile_pool(name="ohst", bufs=3) as ohsp, \
                 tc.tile_pool(name="acc", bufs=1) as accp, \
                 tc.tile_pool(name="wpsum", bufs=6, space="PSUM") as wpp:
                acc_t = accp.tile([128, 128], f32)
                if KABL != "full":
                    nc.vector.memset(acc_t[:], 0.0)
                call_ctr = 0
                for rep in range(KREP):
                    idx_col0 = 0
                    gbuf = None
                    for g in range(4):
                        xg = x[g * gbase:min((g + 1) * gbase, N), :]
                        for w in range(W):
                            if KABL in ("full", "nodve"):
                                psum = wpp.tile([128, 128], f32)
                            for j in range(q):
                                m = w * q + j           # chunk index within group
                                if KABL != "dve" and m % CPC == 0:  # new gather call
                                    nch = min(CPC, per_g - m)
                                    nidx = nch * 128
                                    ixt = ixp.tile([128, CPC * 8], mybir.dt.int16, tag="ix")
                                    nc.sync.dma_start(
                                        ixt[:, :nidx // 16],
                                        idx16[:, idx_col0:idx_col0 + nidx // 16])
                                    if KOH == "ship" and KABL in ("full", "nope"):
                                        k0 = g * per_g + m
                                        ohst = ohsp.tile([128, CPC, 128], odt, tag="oh")
                                        nc.sync.dma_start(
                                            ohst[:, :nch, :],
                                            ohw_p[:].rearrange("p (c e) -> p c e", e=128)[:, k0:k0 + nch, :])
                                    gbuf = gp.tile([128, CPC, 128], gdt, tag="g")
                                    nc.gpsimd.dma_gather(
                                        gbuf[:, :nch, :], xg, ixt[:, :nidx // 16],
                                        nidx, nidx, 128,
                                        single_packet=False,
                                        queue_num=call_ctr % N_QUEUES)
                                    idx_col0 += nidx // 16
                                    call_ctr += 1
                                    if KABL == "gather":
                                        nc.vector.tensor_add(
                                            acc_t[:, 0:1], acc_t[:, 0:1], gbuf[:, 0, 0:1])
                                k = g * per_g + m
                                if KOH == "dve" and KABL in ("full", "nope", "dve"):
                                    oh = ohp.tile([128, 128], f32)
                                    nc.vector.tensor_scalar(
                                        oh[:], iota_t[:],
                                        dsub_t[:, k:k + 1], wv_t[:, k:k + 1],
                                        mybir.AluOpType.is_equal, mybir.AluOpType.mult)
                                if KABL in ("nope", "dve"):
                                    cc = (k % 64) * 2
                                    nc.vector.tensor_add(
                                        acc_t[:, cc:cc + 1], acc_t[:, cc:cc + 1], oh[:, 0:1])
                                if KABL in ("full", "nodve"):
                                    if KABL == "full":
                                        lhsT = (oh[:] if KOH == "dve"
                                                else ohst[:, m % CPC, :])
                                    else:
                                        lhsT = iota_t[:]
                                    nc.tensor.matmul(
                                        psum[:], lhsT, gbuf[:, m % CPC, :],
                                        start=(j == 0), stop=(j == q - 1))
                            if KABL in ("full", "nodve"):
                                if g == 0 and rep == 0:
                                    nc.vector.tensor_copy(agg_t[:, w, :], psum[:])
                                else:
                                    nc.vector.tensor_add(agg_t[:, w, :], agg_t[:, w, :], psum[:])
                if KABL == "dve":
                    for cc in range(0, 128):
                        pass
                if KABL in ("gather", "nope", "dve"):
                    nc.vector.tensor_copy(agg_t[:, 0, :], acc_t[:])

            # ---- post: h = relu(agg @ W1 + b1); pooled; head; softmax ----
            with tc.tile_pool(name="post", bufs=3) as pp, \
                 tc.tile_pool(name="tpsum", bufs=2, space="PSUM") as tpp, \
                 tc.tile_pool(name="hpsum", bufs=2, space="PSUM") as hpp, \
                 tc.tile_pool(name="ppsum", bufs=1, space="PSUM") as ppp:
                pool_psum = ppp.tile([64, G_PER_CORE], f32)
                for w in range(W):
                    tp = tpp.tile([128, 128], f32)
                    nc.tensor.transpose(tp[:], agg_t[:, w, :], ident_t[:])
                    aT = pp.tile([128, 128], f32, tag="aT")
                    nc.vector.tensor_copy(aT[:], tp[:])
                    hp = hpp.tile([128, 64], f32)
                    nc.tensor.matmul(hp[:], aT[:], w1_t[:], start=True, stop=True)
                    h = pp.tile([128, 64], f32, tag="h")
                    nc.vector.tensor_add(h[:], hp[:], b1b_t[:])
                    nc.vector.tensor_scalar_max(h[:], h[:], 0.0)
                    nc.tensor.matmul(pool_psum[:], h[:], pool_t[:, w, :],
                                     start=(w == 0), stop=(w == W - 1))
                pooled = pp.tile([64, G_PER_CORE], f32, tag="pl")
                nc.vector.tensor_mul(pooled[:], pool_psum[:], invc_t[:])
                lg_psum = ppp.tile([G_PER_CORE, 4], f32)
                nc.tensor.matmul(lg_psum[:], pooled[:], w2_t[:], start=True, stop=True)
                lg = pp.tile([G_PER_CORE, 4], f32, tag="lg")
                nc.vector.tensor_add(lg[:], lg_psum[:], b2b_t[:])
                mx = pp.tile([G_PER_CORE, 1], f32, tag="mx")
                nc.vector.reduce_max(mx[:], lg[:], axis=mybir.AxisListType.X)
                nc.vector.tensor_scalar(lg[:], lg[:], mx[:], None,
                                        mybir.AluOpType.subtract)
                ex = pp.tile([G_PER_CORE, 4], f32, tag="ex")
                nc.scalar.activation(ex[:], lg[:], mybir.ActivationFunctionType.Exp)
                sm = pp.tile([G_PER_CORE, 1], f32, tag="sm")
                nc.vector.reduce_sum(sm[:], ex[:], axis=mybir.AxisListType.X)
                rc = pp.tile([G_PER_CORE, 1], f32, tag="rc")
                nc.vector.reciprocal(rc[:], sm[:])
                ot = pp.tile([G_PER_CORE, 4], f32, tag="ot")
                nc.vector.tensor_scalar(ot[:], ex[:], rc[:], None,
                                        mybir.AluOpType.mult)
                nc.sync.dma_start(probs[:], ot[:])
    nc.compile()
    return nc


def kernel(x, edge_src, edge_dst, edge_weight, seg_ids, W1, b1, W2, b2):
    x = np.asarray(x, np.float32)
    in_maps, meta = _prepare(
        x, np.asarray(edge_src), np.asarray(edge_dst),
        np.asarray(edge_weight, np.float32), np.asarray(seg_ids),
        np.asarray(W1, np.float32), np.asarray(b1, np.float32),
        np.asarray(W2, np.float32), np.asarray(b2, np.float32))
    nc = _build_program(meta)
    res = run_bass_kernel_spmd(nc, in_maps, core_ids=list(range(N_CORES)))
    return np.concatenate([res.results[c]["probs"] for c in range(N_CORES)], axis=0)


if __name__ == "__main__":
    pass



# revision 7
# speedup vs baseline: 2.1503x; 2.1503x over previous
"""GNN message passing (GCNConv -> global mean pool -> dense softmax) on 8 TRN2 cores.

Sharding: graphs are partitioned by seg_ids so each core owns 8 whole graphs
(a contiguous node range).  Edges are routed to the core that owns their
destination node; each core gathers source-node features (bf16 x rows, 256B)
straight from HBM with dma_gather, aggregates messages into per-window PSUM
tiles via one-hot matmuls on the TensorEngine (bf16, transposed layout:
psum[w] = [128 x-feats, 128 dst slots]), then per window computes
h = relu(W1.T @ agg + b1), transposes h back and accumulates the pooling
matmul; dense softmax head at the end.  No collectives.

Window-block structure: windows are processed in blocks of WB=8; each
window's PSUM accumulates chunks from all 4 source-groups before a single
eviction, minimizing DVE traffic.  All data-dependent structure (gather
indices, one-hot selector scalars, pooling matrix) is carried in per-core
input arrays so one SPMD program serves all 8 cores.
"""

import os
import sys

sys.path.insert(0, "/opt/trn_rl_repo")

import numpy as np
import ml_dtypes

import concourse.bass as bass
import concourse.bacc as bacc
import concourse.mybir as mybir
import concourse.tile as tile
from concourse.bass_utils import run_bass_kernel_spmd

N_CORES = 8
N_GRAPHS = 64
G_PER_CORE = N_GRAPHS // N_CORES
P = 128          # lanes per chunk / dst slots per window
WB = int(os.environ.get("KWB", "8"))       # windows per block (PSUM residency)
N_QUEUES = int(os.environ.get("KQ", "4"))
KABL = os.environ.get("KABL", "full")      # full | gather | nope
MAX_I16 = 32768
NGROUPS = 4


def _balance_windows(deg4, W):
    """Assign each dst (rows of deg4 [n,4]) to one of W windows, <=128 dsts per
    window, minimizing the max per-(group,window) edge load."""
    n = deg4.shape[0]
    order = np.argsort(-deg4.sum(1), kind="stable")
    load = np.zeros((W, NGROUPS), np.int64)
    slots = np.zeros(W, np.int64)
    win = np.zeros(n, np.int64)
    for d in order:
        free = slots < P
        cand = (load[free] + deg4[d]).max(1)
        wsel = np.flatnonzero(free)[np.argmin(cand)]
        win[d] = wsel
        load[wsel] += deg4[d]
        slots[wsel] += 1
    return win, load


def _prep_core(es, dl, ew, n_loc, W, gbase):
    """Per-core schedule: window/slot assignment for each local dst."""
    g = (es // gbase).astype(np.int64)
    deg4 = np.zeros((max(n_loc, 1), NGROUPS), np.int64)
    np.add.at(deg4, (dl, g), 1)
    win, load = _balance_windows(deg4, W)
    slot_of = np.zeros(max(n_loc, 1), np.int64)
    for w in range(W):
        members = np.flatnonzero(win == w)
        slot_of[members] = np.arange(len(members))
    row_of = win * P + slot_of
    q_req = int(np.ceil(load.max() / P)) if load.size else 1
    return {
        "g": g, "win": win[dl], "slot": slot_of[dl], "row_of": row_of,
        "q_req": max(q_req, 1),
    }


def _wb_sizes(W):
    nwb = (W + WB - 1) // WB
    return [min(WB, W - b * WB) for b in range(nwb)]


def _col_of(win, g, q, W):
    """Global chunk-column base index for (window, group) under the
    (wb, g, wi, j) enumeration order."""
    sizes = _wb_sizes(W)
    base = np.zeros(len(sizes) + 1, np.int64)
    for b, nw in enumerate(sizes):
        base[b + 1] = base[b] + NGROUPS * nw * q
    wb = win // WB
    wi = win % WB
    nw = np.asarray(sizes)[wb]
    return base[wb] + g * nw * q + wi * q


def _build_schedule(core, q, W):
    """Fill offs16 / dsub / wv [P, C] arrays for one core."""
    g, win, slot = core["g"], core["win"], core["slot"]
    C = NGROUPS * W * q
    offs = np.zeros((P, C), np.int16)
    dsub = np.full((P, C), -1.0, np.float32)
    wv = np.zeros((P, C), np.float32)
    # order edges by (g, w, slot) to get stable positions within (g,w) blocks
    key = (g * W + win) * P + slot
    order = np.argsort(key, kind="stable")
    gs, ws = g[order], win[order]
    blk = gs * W + ws
    changes = np.r_[True, blk[1:] != blk[:-1]]
    block_start = np.maximum.accumulate(np.where(changes, np.arange(len(blk)), 0))
    pos = np.arange(len(blk)) - block_start
    if np.any(pos // P >= q):
        raise RuntimeError("window overflow: q too small")
    k = _col_of(ws, gs, q, W) + pos // P
    p = pos % P
    offs[p, k] = (core["es_sorted"][order] - gs * core["gbase"]).astype(np.int16)
    dsub[p, k] = slot[order].astype(np.float32)
    wv[p, k] = core["ew_sorted"][order]
    return offs, dsub, wv


def _wrap_idx(offs, W, q):
    """Pack per-call int16 index lists in the HW wrap-16 layout, 8x replicated.
    One call per (wb, g) spanning nw*q chunks.  Returns [128, C*8]."""
    cols = []
    k0 = 0
    for nw in _wb_sizes(W):
        for g in range(NGROUPS):
            nch = nw * q
            ii = offs[:, k0:k0 + nch].T.reshape(-1)  # i = chunk*128 + p
            cols.append(ii.reshape(-1, 16).T)        # [16, n/16]
            k0 += nch
    arr = np.concatenate(cols, axis=1)
    return np.tile(arr, (8, 1)).astype(np.int16)


def _prepare(x, edge_src, edge_dst, edge_weight, seg_ids, W1, b1, W2, b2):
    N = x.shape[0]
    gbase = int(np.ceil(N / NGROUPS))
    assert gbase <= MAX_I16
    bounds = np.searchsorted(seg_ids, np.arange(0, N_GRAPHS + 1, G_PER_CORE))
    n_locs = np.diff(bounds)
    W = int(np.ceil(n_locs.max() / P))
    core_of_edge = np.searchsorted(bounds, edge_dst, side="right") - 1

    cores = []
    for c in range(N_CORES):
        m = core_of_edge == c
        es, ed, ew = edge_src[m], edge_dst[m] - bounds[c], edge_weight[m]
        info = _prep_core(es, ed, ew, int(n_locs[c]), W, gbase)
        info.update(es_sorted=es, ew_sorted=ew, gbase=gbase)
        cores.append(info)
    q = max(ci["q_req"] for ci in cores)
    C = NGROUPS * W * q

    iota = np.tile(np.arange(P, dtype=np.float32), (P, 1)).astype(ml_dtypes.bfloat16)
    ident = np.eye(P, dtype=np.float32).astype(ml_dtypes.bfloat16)
    b1c = b1.reshape(64, 1).astype(np.float32)
    b2b = np.tile(b2[None, :], (G_PER_CORE, 1)).astype(np.float32)
    x_bf = np.ascontiguousarray(x.astype(ml_dtypes.bfloat16))

    in_maps = []
    for c in range(N_CORES):
        ci = cores[c]
        offs, dsub, wv = _build_schedule(ci, q, W)
        idx16 = _wrap_idx(offs, W, q)
        # pooling matrix in SBUF layout [128, W * G_PER_CORE]
        pool = np.zeros((P, W, G_PER_CORE), np.float32)
        segs_loc = seg_ids[bounds[c]:bounds[c + 1]] - c * G_PER_CORE
        rows = ci["row_of"][:n_locs[c]]
        pool[rows % P, rows // P, segs_loc] = 1.0
        cnts = np.bincount(segs_loc, minlength=G_PER_CORE).astype(np.float32)
        invc = np.tile((1.0 / np.maximum(cnts, 1.0))[None, :], (64, 1)).astype(np.float32)
        in_maps.append({
            "x": x_bf,
            "idx16": idx16,
            "dsub": dsub, "wv": wv,
            "pool": pool.reshape(P, W * G_PER_CORE).astype(ml_dtypes.bfloat16),
            "invc": invc,
            "W1": np.ascontiguousarray(W1.astype(ml_dtypes.bfloat16)),
            "b1c": b1c,
            "W2": np.ascontiguousarray(W2, np.float32),
            "b2b": b2b,
            "iota": iota, "ident": ident,
        })
    meta = {"N": N, "W": W, "q": q, "C": C, "gbase": gbase,
            "idx_cols": in_maps[0]["idx16"].shape[1]}
    return in_maps, meta


def _build_program(meta):
    N, W, q, gbase = meta["N"], meta["W"], meta["q"], meta["gbase"]
    C = meta["C"]
    f32 = mybir.dt.float32
    bf16 = mybir.dt.bfloat16
    nc = bacc.Bacc("TRN2", target_bir_lowering=False, debug=False,
                   num_devices=N_CORES, num_swdge_queues=N_QUEUES)
    x = nc.declare_dram_parameter("x", [N, 128], bf16, isOutput=False)
    idx16 = nc.declare_dram_parameter("idx16", [128, meta["idx_cols"]], mybir.dt.int16, isOutput=False)
    dsub = nc.declare_dram_parameter("dsub", [128, C], f32, isOutput=False)
    wv = nc.declare_dram_parameter("wv", [128, C], f32, isOutput=False)
    pool_p = nc.declare_dram_parameter("pool", [128, W * G_PER_CORE], bf16, isOutput=False)
    invc_p = nc.declare_dram_parameter("invc", [64, G_PER_CORE], f32, isOutput=False)
    W1_p = nc.declare_dram_parameter("W1", [128, 64], bf16, isOutput=False)
    b1c_p = nc.declare_dram_parameter("b1c", [64, 1], f32, isOutput=False)
    W2_p = nc.declare_dram_parameter("W2", [64, 4], f32, isOutput=False)
    b2b_p = nc.declare_dram_parameter("b2b", [G_PER_CORE, 4], f32, isOutput=False)
    iota_p = nc.declare_dram_parameter("iota", [128, 128], bf16, isOutput=False)
    ident_p = nc.declare_dram_parameter("ident", [128, 128], bf16, isOutput=False)
    probs = nc.declare_dram_parameter("probs", [G_PER_CORE, 4], f32, isOutput=True)

    wb_sizes = _wb_sizes(W)
    with tile.TileContext(nc) as tc:
        with tc.tile_pool(name="const", bufs=1) as cp, \
             tc.tile_pool(name="stream", bufs=1) as sp:
            iota_t = cp.tile([128, 128], bf16)
            nc.sync.dma_start(iota_t[:], iota_p[:])
            ident_t = cp.tile([128, 128], bf16)
            nc.sync.dma_start(ident_t[:], ident_p[:])
            w1_t = cp.tile([128, 64], bf16)
            nc.sync.dma_start(w1_t[:], W1_p[:])
            b1c_t = cp.tile([64, 1], f32)
            nc.sync.dma_start(b1c_t[:], b1c_p[:])
            w2_t = cp.tile([64, 4], f32)
            nc.sync.dma_start(w2_t[:], W2_p[:])
            b2b_t = cp.tile([G_PER_CORE, 4], f32)
            nc.sync.dma_start(b2b_t[:], b2b_p[:])
            invc_t = cp.tile([64, G_PER_CORE], f32)
            nc.sync.dma_start(invc_t[:], invc_p[:])
            pool_t = cp.tile([128, W, G_PER_CORE], bf16)
            nc.sync.dma_start(pool_t[:], pool_p[:].rearrange("p (w f) -> p w f", f=G_PER_CORE))
            dsub_t = sp.tile([128, C], f32)
            nc.sync.dma_start(dsub_t[:], dsub[:])
            wv_t = sp.tile([128, C], f32)
            nc.sync.dma_start(wv_t[:], wv[:])

            with tc.tile_pool(name="gbuf", bufs=int(os.environ.get("KGB", "8"))) as gp, \
                 tc.tile_pool(name="idxs", bufs=8) as ixp, \
                 tc.tile_pool(name="oh", bufs=8) as ohp, \
                 tc.tile_pool(name="post", bufs=3) as pp, \
                 tc.tile_pool(name="acc", bufs=1) as accp, \
                 tc.tile_pool(name="wpsum", bufs=4, space="PSUM") as wpp, \
                 tc.tile_pool(name="hppsum", bufs=1, space="PSUM") as hpp, \
                 tc.tile_pool(name="htpsum", bufs=1, space="PSUM") as htp, \
                 tc.tile_pool(name="ppsum", bufs=1, space="PSUM") as ppp:
                pool_ps = ppp.tile([64, G_PER_CORE], f32)
                acc_t = None
                if KABL != "full":
                    acc_t = accp.tile([128, 128], f32)
                    nc.vector.memset(acc_t[:], 0.0)
                call_ctr = 0
                kcol = 0
                idx_col0 = 0
                wb_base = 0
                for wb, nw in enumerate(wb_sizes):
                    nquad = (nw + 3) // 4
                    quads = [wpp.tile([128, 4, 128], f32, tag="wps",
                                      name=f"wps{wb}_{qi}") for qi in range(nquad)]
                    nch = nw * q
                    nidx = nch * 128
                    gbufs = []
                    for g in range(NGROUPS):
                        xg = x[g * gbase:min((g + 1) * gbase, N), :]
                        ixt = ixp.tile([128, WB * q * 8], mybir.dt.int16, tag="ix")
                        nc.sync.dma_start(
                            ixt[:, :nidx // 16],
                            idx16[:, idx_col0:idx_col0 + nidx // 16])
                        gbuf = gp.tile([128, WB * q, 128], bf16, tag="g",
                                       name=f"g{wb}_{g}")
                        nc.gpsimd.dma_gather(
                            gbuf[:, :nch, :], xg, ixt[:, :nidx // 16],
                            nidx, nidx, 128,
                            single_packet=False,
                            queue_num=call_ctr % N_QUEUES)
                        gbufs.append(gbuf)
                        idx_col0 += nidx // 16
                        call_ctr += 1
                        if KABL == "gather":
                            nc.vector.tensor_add(
                                acc_t[:, 0:1], acc_t[:, 0:1], gbuf[:, 0, 0:1])
                    if KABL == "gather":
                        wb_base += NGROUPS * nch
                        continue
                    # one window's accumulation group runs to completion before
                    # the next window in the same PSUM bank (start= clears the
                    # whole bank's has_written bits)
                    for wi in range(nw):
                        for g in range(NGROUPS):
                            for j in range(q):
                                k = wb_base + g * nw * q + wi * q + j
                                oh = ohp.tile([128, 128], bf16)
                                nc.vector.tensor_scalar(
                                    oh[:], iota_t[:],
                                    dsub_t[:, k:k + 1], wv_t[:, k:k + 1],
                                    mybir.AluOpType.is_equal, mybir.AluOpType.mult)
                                if KABL == "nope":
                                    nc.vector.tensor_add(
                                        acc_t[:, 0:1], acc_t[:, 0:1], oh[:, 0:1])
                                    continue
                                nc.tensor.matmul(
                                    quads[wi // 4][:, wi % 4, :],
                                    gbufs[g][:, wi * q + j, :], oh[:],
                                    start=(g == 0 and j == 0),
                                    stop=(g == NGROUPS - 1 and j == q - 1))
                    wb_base += NGROUPS * nch
                    if KABL != "full":
                        continue
                    # ---- post per quad: h = relu(W1.T @ agg + b1); pool ----
                    for qi in range(nquad):
                        nqw = min(4, nw - qi * 4)
                        nf = nqw * 128
                        aggT = pp.tile([128, 4 * 128], bf16, tag="aggT")
                        nc.vector.tensor_copy(
                            aggT[:, :nf],
                            quads[qi][:].rearrange("p a b -> p (a b)")[:, :nf])
                        hp = hpp.tile([64, 4 * 128], f32, tag="hp")
                        nc.tensor.matmul(hp[:, :nf], w1_t[:], aggT[:, :nf],
                                         start=True, stop=True)
                        hT = pp.tile([64, 4 * 128], bf16, tag="hT")
                        nc.vector.tensor_scalar(
                            hT[:, :nf], hp[:, :nf], b1c_t[:], 0.0,
                            mybir.AluOpType.add, mybir.AluOpType.max)
                        hps = htp.tile([128, 4, 64], bf16, tag="hps")
                        for wi2 in range(nqw):
                            nc.tensor.transpose(
                                hps[:, wi2, :],
                                hT[:, wi2 * 128:(wi2 + 1) * 128],
                                ident_t[:64, :64])
                        h = pp.tile([128, 4, 64], bf16, tag="h")
                        nc.vector.tensor_copy(
                            h[:].rearrange("p a b -> p (a b)")[:, :nqw * 64],
                            hps[:].rearrange("p a b -> p (a b)")[:, :nqw * 64])
                        for wi2 in range(nqw):
                            w = wb * WB + qi * 4 + wi2
                            nc.tensor.matmul(pool_ps[:], h[:, wi2, :],
                                             pool_t[:, w, :],
                                             start=(w == 0), stop=(w == W - 1))

                # ---- head: pooled mean; logits; softmax ----
                if KABL == "full":
                    pooled = pp.tile([64, G_PER_CORE], f32, tag="pl")
                    nc.vector.tensor_mul(pooled[:], pool_ps[:], invc_t[:])
                else:
                    pooled = pp.tile([64, G_PER_CORE], f32, tag="pl")
                    nc.vector.tensor_copy(pooled[:], acc_t[:64, :G_PER_CORE])
                lg_psum = ppp.tile([G_PER_CORE, 4], f32)
                nc.tensor.matmul(lg_psum[:], pooled[:], w2_t[:], start=True, stop=True)
                lg = pp.tile([G_PER_CORE, 4], f32, tag="lg")
                nc.vector.tensor_add(lg[:], lg_psum[:], b2b_t[:])
                mx = pp.tile([G_PER_CORE, 1], f32, tag="mx")
                nc.vector.reduce_max(mx[:], lg[:], axis=mybir.AxisListType.X)
                nc.vector.tensor_scalar(lg[:], lg[:], mx[:], None,
                                        mybir.AluOpType.subtract)
                ex = pp.tile([G_PER_CORE, 4], f32, tag="ex")
                nc.scalar.activation(ex[:], lg[:], mybir.ActivationFunctionType.Exp)
                sm = pp.tile([G_PER_CORE, 1], f32, tag="sm")
                nc.vector.reduce_sum(sm[:], ex[:], axis=mybir.AxisListType.X)
                rc = pp.tile([G_PER_CORE, 1], f32, tag="rc")
                nc.vector.reciprocal(rc[:], sm[:])
                ot = pp.tile([G_PER_CORE, 4], f32, tag="ot")
                nc.vector.tensor_scalar(ot[:], ex[:], rc[:], None,
                                        mybir.AluOpType.mult)
                nc.sync.dma_start(probs[:], ot[:])
    nc.compile()
    return nc


def kernel(x, edge_src, edge_dst, edge_weight, seg_ids, W1, b1, W2, b2):
    x = np.asarray(x, np.float32)
    in_maps, meta = _prepare(
        x, np.asarray(edge_src), np.asarray(edge_dst),
        np.asarray(edge_weight, np.float32), np.asarray(seg_ids),
        np.asarray(W1, np.float32), np.asarray(b1, np.float32),
        np.asarray(W2, np.float32), np.asarray(b2, np.float32))
    nc = _build_program(meta)
    res = run_bass_kernel_spmd(nc, in_maps, core_ids=list(range(N_CORES)))
    return np.concatenate([res.results[c]["probs"] for c in range(N_CORES)], axis=0)


if __name__ == "__main__":
    pass
